# revision 11
# baseline (speedup 1.0000x reference)
"""Trainium2 Bass kernel for ContractiveInvertibleGNN feed-forward.

Math (reference, group_mask == I_32), rewritten in relu form so each
residual-block activation needs exactly ONE engine pass:
  lrelu(z) = alpha*z + (1-alpha)*relu(z)
g phase, per node j (column-major [128 hidden, 2048 batch] tiles):
  h1  = lrelu(x_j*U_j + C1_j)                       (Act fused / DVE+Pool)
  pa  = GW2^T h1                                    (PE, psum)
  t2r = relu(pa + g_b2)                (one Act-Relu / DVE tensor_scalar)
  X_emb = G4^T h1 + ((1-al)*GW3)^T t2r + const      G4 = GW3 + al*GW2@GW3
f phase, per node i:
  w1  = FW1^T X_aggr + C2_i ; hf1 = lrelu(w1)       (Act fused)
  zf  = FW2^T hf1 ; tfr = relu(zf + f_b2)           (one pass)
  out = V4^T hf1 + ((1-al)*V)^T tfr + const         V4 = V + al*FW2@V
All alpha*z linear terms and bias constants are folded host-side into
G4/V4/C2/f_b3, so the relu passes are exact (no alpha approximation).

Dtypes: activations bf16 except t2r (fp8 e4m3; half its entries are exact
zeros); the t2r aggregation into
X_emb runs as fp8 DoubleRow matmuls pairing two chunks per pass (disjoint
output row groups), all other matmuls bf16 except FW1 which consumes the
aggregation output transposed straight out of PSUM as f32 (float32r
matmul), skipping a psum->sbuf convert pass.

Aggregation: Xe[(c,d),(j,t)] --DVE T1--> Xt[(c,j),(t,d)] --kron(I4,W_adj)
matmul--> psum --DVE T2 (psum source)--> Xa[(c,d),(i,t)] f32.

Sharding: pure data-parallel over batch across 8 cores (2048 rows each).
"""

import os
import sys

import numpy as np

for _p in ("/opt/trn_rl_repo", "/root/.axon_site/_ro/trn_rl_repo"):
    if os.path.isdir(_p) and _p not in sys.path:
        sys.path.insert(0, _p)

N = 32          # nodes
D = 32          # processed dim (== N, group_mask = I)
A = 128         # hidden width
B = 16384       # batch
NCORES = 8
BC = B // NCORES        # 2048 rows per core
CH = 512                # matmul free-dim chunk
NCH = BC // CH          # 4 chunks (partition-group stacking factor)
ALPHA = 0.01
AF = ALPHA            # weight of the folded linear term
BF = 1.0 - ALPHA      # weight of the relu carrier
NVP = 8                 # output nodes sharing one V-dot psum tile

# engine split tunables (columns handled by Act; rest DVE/Pool)
H1A = 1024              # h1 cols on Act (rest: DVE z + Pool STT lrelu)
T2AA = 1024             # t2a cols (of 2048) on Act-Abs (rest DVE)
TFAA = 1024             # tfa cols on Act-Abs (rest DVE)
HCH = 2 * CH            # 1024


def _build_program():
    from contextlib import ExitStack

    from concourse import bacc, mybir, tile

    f32 = mybir.dt.float32
    f32r = mybir.dt.float32r
    bf16 = mybir.dt.bfloat16
    fp8 = mybir.dt.float8e4
    LRELU = mybir.ActivationFunctionType.Lrelu
    RELU = mybir.ActivationFunctionType.Relu
    ALU_MULT = mybir.AluOpType.mult
    ALU_ADD = mybir.AluOpType.add
    ALU_MAX = mybir.AluOpType.max
    ALU_MIN = mybir.AluOpType.min
    DR = mybir.MatmulPerfMode.DoubleRow

    nc = bacc.Bacc("TRN2", target_bir_lowering=False, debug=False)

    def din(name, shape, dt):
        return nc.dram_tensor(
            name, list(shape), dt, kind="ExternalInput"
        ).ap()

    xt_d = din("XT", (N, BC), bf16)
    gw2_d = din("GW2", (A, A), bf16)
    fw2_d = din("FW2", (A, A), bf16)
    g5p_d = din("G5P", (A, NCH * A), bf16)     # col-block c: G5 at cols 32c..
    g3d_d = din("G3D", (A, 2 * 2 * A), fp8)    # (q, slot, m): b*GW3 routed
    fw1p_d = din("FW1P", (A, NCH * A), bf16)   # row-block c: f_W1[:32] rows 32c..
    bd_d = din("BD", (A, A), bf16)             # kron(I4, W_adj)
    u_d = din("U", (A, N), f32)
    c1_d = din("C1", (A, N), f32)
    c2_d = din("C2", (A, N), f32)
    gb2_d = din("GB2", (A, 1), f32)
    fb2_d = din("FB2", (A, 1), f32)
    # V-dot stationaries: slice (i,c) = cols [(i*NCH+c)*D, +D) with the vector
    # at column NCH*(i%NVP)+c so NVP nodes' dots accumulate into one psum tile.
    v5p_d = din("V5P", (A, N * NCH * D), bf16)   # V5 = V + a*FW2@V
    vbp_d = din("VBP", (A, N * NCH * D), bf16)   # b*V
    out_d = nc.dram_tensor("OUT", [N, BC], f32, kind="ExternalOutput").ap()

    H1T = BC - H1A          # h1 tail cols (DVE z + Pool STT path)
    T2AH = T2AA // 2        # Act-abs cols per half
    TFAH = TFAA // 2

    with tile.TileContext(nc) as tc, ExitStack() as ctx:
        const = ctx.enter_context(tc.tile_pool(name="const", bufs=1))
        xep = ctx.enter_context(tc.tile_pool(name="xep", bufs=1))
        workp = ctx.enter_context(tc.tile_pool(name="work", bufs=2))
        scrp = ctx.enter_context(tc.tile_pool(name="scr", bufs=3))
        outp = ctx.enter_context(tc.tile_pool(name="outs", bufs=1))
        # PSUM: ppA 3x [A,1024] (6 banks) + ppB 2x [A,512] (2 banks) = 8.
        ppA = ctx.enter_context(tc.tile_pool(name="ppA", bufs=3, space="PSUM"))
        ppB = ctx.enter_context(tc.tile_pool(name="ppB", bufs=2, space="PSUM"))

        def load_const(ap_dram, shape):
            t = const.tile(list(shape), ap_dram.dtype,
                           tag=f"c_{ap_dram.tensor.name}")
            nc.sync.dma_start(t[:, :], ap_dram)
            return t

        gw2_s = load_const(gw2_d, (A, A))
        fw2_s = load_const(fw2_d, (A, A))
        g5p_s = load_const(g5p_d, (A, NCH * A))
        g3d_s = load_const(g3d_d, (A, 2 * 2 * A))
        fw1p_s = load_const(fw1p_d, (A, NCH * A))
        bd_s = load_const(bd_d, (A, A))
        u_s = load_const(u_d, (A, N))
        c1_s = load_const(c1_d, (A, N))
        c2_s = load_const(c2_d, (A, N))
        gb2_s = load_const(gb2_d, (A, 1))
        fb2_s = load_const(fb2_d, (A, 1))
        v5p_s = load_const(v5p_d, (A, N * NCH * D))
        vbp_s = load_const(vbp_d, (A, N * NCH * D))

        # Xe[(c,d), (j,t)] = X_emb[d, j, c*CH+t]  (bf16)
        xe = xep.tile([A, N * CH], bf16, tag="xe")

        # ---------------- g phase ----------------
        # Software-pipelined one node deep: DMA 3 ahead, h1 2 ahead, so each
        # engine's queue head is always ready work.
        xbc_tiles = {}
        h1_tiles = {}
        pm3_prev = {}

        def emit_xbc(j):
            xbc = workp.tile([A, BC], bf16, tag="xbc", bufs=3)
            nc.sync.dma_start(
                xbc[:, :], xt_d[j : j + 1, :].partition_broadcast(A)
            )
            xbc_tiles[j] = xbc

        def emit_h1(j):
            xbc = xbc_tiles.pop(j)
            h1 = workp.tile([A, BC], bf16, tag="h1", bufs=3)
            nc.scalar.activation(
                h1[:, :H1A], xbc[:, :H1A], LRELU,
                bias=c1_s[:, j : j + 1], scale=u_s[:, j : j + 1], alpha=ALPHA,
            )
            if H1T:
                # DVE: z = x*u + c1 (bf16, 4x mode); Pool: n = (a-1)*min(z,0)
                # (no PSUM access on Pool); DVE: h1 = z + n.
                zt = scrp.tile([A, H1T], bf16, tag="zt", bufs=2)
                mt = scrp.tile([A, H1T], bf16, tag="mt", bufs=2)
                nc.vector.tensor_scalar(zt[:, :], xbc[:, H1A:],
                                        u_s[:, j : j + 1], c1_s[:, j : j + 1],
                                        ALU_MULT, ALU_ADD)
                nc.gpsimd.tensor_scalar(mt[:, :], zt[:, :], 0.0, ALPHA - 1.0,
                                        ALU_MIN, ALU_MULT)
                nc.vector.tensor_tensor(h1[:, H1A:], zt[:, :], mt[:, :],
                                        ALU_ADD)
            h1_tiles[j] = h1

        def xe_copy(jj):
            # X_emb psum -> xe bf16 (alternate Act/DVE by parity for balance)
            pm3 = pm3_prev.pop(jj)
            dst = xe[:, jj * CH : (jj + 1) * CH]
            if jj % 4 == 0:
                nc.scalar.copy(dst, pm3[:, :])
            else:
                nc.vector.tensor_copy(dst, pm3[:, :])

        emit_xbc(0)
        emit_xbc(1)
        emit_xbc(2)
        emit_h1(0)
        emit_h1(1)

        for j in range(N):
            if j > 1:
                xe_copy(j - 2)
            if j + 3 < N:
                emit_xbc(j + 3)
            if j + 2 < N:
                emit_h1(j + 2)
            h1 = h1_tiles.pop(j)
            t2a = workp.tile([A, BC], fp8, tag="t2a", bufs=2)
            for h in range(2):  # halves of 1024 cols
                pa = ppA.tile([A, HCH], f32, tag="pA", name=f"pa_{j}_{h}")
                for q in range(2):
                    sl = slice(h * HCH + q * CH, h * HCH + (q + 1) * CH)
                    nc.tensor.matmul(
                        pa[:, q * CH : (q + 1) * CH], gw2_s[:, :],
                        h1[:, sl], start=True, stop=True,
                    )
                off = h * HCH
                if T2AH:
                    # Act: relu(pa + g_b2), bias fused
                    nc.scalar.activation(
                        t2a[:, off : off + T2AH], pa[:, :T2AH], RELU,
                        bias=gb2_s[:, 0:1],
                    )
                if T2AH < HCH:
                    # DVE one-pass relu: (pa + b2) max 0
                    nc.vector.tensor_scalar(
                        t2a[:, off + T2AH : off + HCH], pa[:, T2AH:],
                        gb2_s[:, 0:1], 0.0, ALU_ADD, ALU_MAX)
            # X_emb = G5^T h1 (4 bf16 mms) + (b GW3)^T t2a (2 fp8 DR mms,
            # each pairing chunks (q, q+2) into disjoint psum row groups).
            pm3 = ppB.tile([A, CH], f32, tag="pB", name=f"pm3_{j}")
            for c in range(NCH):
                nc.tensor.matmul(pm3[:, :], g5p_s[:, c * A : (c + 1) * A],
                                 h1[:, c * CH : (c + 1) * CH],
                                 start=(c == 0), stop=False)
            t2a3 = t2a.rearrange("p (two t) -> p two t", two=2)
            for q in range(2):
                g3 = g3d_s[:, q * 2 * A : (q + 1) * 2 * A].rearrange(
                    "p (two m) -> p two m", two=2)
                nc.tensor.matmul(pm3[:, :], g3,
                                 t2a3[:, :, q * CH : (q + 1) * CH],
                                 start=False, stop=(q == 1), perf_mode=DR)
            pm3_prev[j] = pm3
        xe_copy(N - 2)
        xe_copy(N - 1)

        # ---------------- aggregation ----------------
        # T1: Xe[(c,d),(j,t)] -> Xt[(c,j),(t,d)]   (DVE 32x32 blocks)
        xt3 = xe.rearrange("p (j t) -> p j t", j=N).transpose([0, 2, 1])
        xtile = xep.tile([A, CH * D], bf16, tag="xt")
        xto = xtile.rearrange("p (t d) -> p t d", d=D)
        TS = 8
        tstep = CH // TS
        for s in range(TS):
            nc.vector.transpose(
                xto[:, s * tstep : (s + 1) * tstep, :],
                xt3[:, s * tstep : (s + 1) * tstep, :],
            )
        # agg windows + T2 back, transposing straight out of psum into f32 xa
        xa = xep.tile([A, N * CH], bf16, tag="xa")
        xa3 = xa.rearrange("p (i t) -> p i t", i=N).transpose([0, 2, 1])
        WT = HCH // D  # 32 t per window
        for w in range(CH // WT):  # 16 windows of 1024 cols
            pg = ppA.tile([A, HCH], f32, tag="pA", name=f"pg_{w}")
            for q in range(2):
                nc.tensor.matmul(
                    pg[:, q * CH : (q + 1) * CH], bd_s[:, :],
                    xtile[:, w * HCH + q * CH : w * HCH + (q + 1) * CH],
                    start=True, stop=True,
                )
            # psum -> bf16 convert (alternating Act/DVE), then 32x32 T2
            xcv = scrp.tile([A, HCH], bf16, tag="xcv", bufs=2)
            if w % 2 == 0:
                nc.scalar.copy(xcv[:, :], pg[:, :])
            else:
                nc.vector.tensor_copy(xcv[:, :], pg[:, :])
            nc.vector.transpose(
                xa3[:, w * WT : (w + 1) * WT, :],
                xcv.rearrange("p (t i) -> p t i", i=D)[:, :, :],
            )

        # ---------------- f phase ----------------
        # Per node i: FW1 (f32r over xa) -> hf1 (Act lrelu) -> FW2 -> tfa
        # (one-pass |.|) -> V-dots (8 bf16 mms into shared pr tile).
        hf1_tiles = {}
        tfa_tiles = {}
        pw_tiles = {}
        pz_tiles = {}
        vdot_state = {"pr": None}

        def emit_fw1(i):
            rhs = xa[:, i * CH : (i + 1) * CH]
            for h in range(2):
                pw = ppA.tile([A, HCH], f32, tag="pA", name=f"pw_{i}_{h}")
                for q in range(2):
                    c = h * 2 + q
                    nc.tensor.matmul(
                        pw[:, q * CH : (q + 1) * CH],
                        fw1p_s[:, c * A : (c + 1) * A], rhs,
                        start=True, stop=True,
                    )
                pw_tiles[(i, h)] = pw

        def emit_hf1(i):
            hf1 = workp.tile([A, BC], bf16, tag="hf1", bufs=3)
            for h in range(2):
                pw = pw_tiles.pop((i, h))
                nc.scalar.activation(
                    hf1[:, h * HCH : (h + 1) * HCH], pw[:, :], LRELU,
                    bias=c2_s[:, i : i + 1], alpha=ALPHA,
                )
            hf1_tiles[i] = hf1

        def emit_fw2(i):
            hf1 = hf1_tiles[i]
            for h in range(2):
                pz = ppA.tile([A, HCH], f32, tag="pA", name=f"pz_{i}_{h}")
                for q in range(2):
                    c = h * 2 + q
                    nc.tensor.matmul(
                        pz[:, q * CH : (q + 1) * CH], fw2_s[:, :],
                        hf1[:, c * CH : (c + 1) * CH], start=True, stop=True,
                    )
                pz_tiles[(i, h)] = pz

        def emit_tfa(i):
            tfa = workp.tile([A, BC], bf16, tag="tfa", bufs=2)
            for h in range(2):
                pz = pz_tiles.pop((i, h))
                off = h * HCH
                if TFAH:
                    nc.scalar.activation(
                        tfa[:, off : off + TFAH], pz[:, :TFAH], RELU,
                        bias=fb2_s[:, 0:1],
                    )
                if TFAH < HCH:
                    nc.vector.tensor_scalar(
                        tfa[:, off + TFAH : off + HCH], pz[:, TFAH:],
                        fb2_s[:, 0:1], 0.0, ALU_ADD, ALU_MAX)
            tfa_tiles[i] = tfa

        def emit_vdot(i):
            hf1 = hf1_tiles.pop(i)
            tfa = tfa_tiles.pop(i)
            if i % NVP == 0:
                vdot_state["pr"] = ppB.tile([A, CH], f32, tag="pB",
                                            name=f"pr_{i}")
            pr = vdot_state["pr"]
            for c in range(NCH):
                base = (i * NCH + c) * D
                st = (i % NVP == 0 and c == 0)
                sp = (i % NVP == NVP - 1 and c == NCH - 1)
                nc.tensor.matmul(pr[:D, :], v5p_s[:, base : base + D],
                                 hf1[:, c * CH : (c + 1) * CH],
                                 start=st, stop=False)
                nc.tensor.matmul(pr[:D, :], vbp_s[:, base : base + D],
                                 tfa[:, c * CH : (c + 1) * CH],
                                 start=False, stop=sp)
            if i % NVP == NVP - 1:
                g0 = i - (NVP - 1)
                osb = outp.tile([NVP * NCH, CH], f32, tag="o")
                nc.vector.tensor_copy(osb[:, :], pr[: NVP * NCH, :])
                nc.sync.dma_start(
                    out_d[g0 : g0 + NVP, :].rearrange(
                        "o (c t) -> (o c) t", c=NCH),
                    osb[:, :],
                )

        # pipeline: fw1 two ahead, fw2 one ahead
        emit_fw1(0)
        emit_hf1(0)
        emit_fw1(1)
        for i in range(N):
            emit_fw2(i)
            if i + 1 < N:
                emit_hf1(i + 1)
            if i + 2 < N:
                emit_fw1(i + 2)
            emit_tfa(i)
            emit_vdot(i)

    nc.compile()
    return nc


_NC_CACHE = {}


def _get_program(zero_b2=True):
    if "nc" not in _NC_CACHE:
        _NC_CACHE["nc"] = _build_program()
    return _NC_CACHE["nc"]


def _bf16(x):
    import ml_dtypes
    return np.asarray(x, np.float32).astype(ml_dtypes.bfloat16)


def _fp8(x):
    import ml_dtypes
    return np.asarray(x, np.float32).astype(ml_dtypes.float8_e4m3)


def _host_consts(W, embeddings, g_W1, g_b1, g_W2, g_b2, g_W3, g_b3,
                 f_W1, f_b1, f_W2, f_b2, f_W3, f_b3):
    f = np.float32
    W_adj = (W * (1.0 - np.eye(N, dtype=f))).astype(f)
    U = np.ascontiguousarray(g_W1[:D].T, dtype=f)                    # [A, N]
    C1 = np.ascontiguousarray((embeddings @ g_W1[D:] + g_b1).T, f)   # [A, N]
    s = W_adj.sum(axis=0)                                            # [N]
    # X_emb constant part: kappa = g_b3 + a * GW3^T g_b2  (per output dim)
    kappa = g_b3 + AF * (g_b2 @ g_W3)
    C2 = (embeddings @ f_W1[D:] + f_b1 + np.outer(s, kappa @ f_W1[:D]))
    C2 = np.ascontiguousarray(C2.T, dtype=f)                         # [A, N]
    G5 = (g_W3 + AF * g_W2 @ g_W3).astype(f)                         # [A, D]
    G5P = np.zeros((A, NCH * A), f)
    FW1P = np.zeros((A, NCH * A), f)
    for c in range(NCH):
        G5P[:, c * A + c * D : c * A + (c + 1) * D] = G5
        FW1P[c * D : (c + 1) * D, c * A : (c + 1) * A] = f_W1[:D]
    # t2a DoubleRow stationaries: q-th mm pairs chunks (q, q+2)
    G3 = (BF * g_W3).astype(f)
    G3D = np.zeros((A, 2, 2, A), f)
    for q in range(2):
        for slot in range(2):
            c = q + 2 * slot
            G3D[:, q, slot, c * D : (c + 1) * D] = G3
    BD = np.kron(np.eye(NCH, dtype=f), W_adj).astype(f)
    V5 = (f_W3 + AF * f_W2 @ f_W3).astype(f)                         # [A, D]
    VB = (BF * f_W3).astype(f)
    V5P = np.zeros((A, N * NCH * D), f)
    VBP = np.zeros((A, N * NCH * D), f)
    for i in range(N):
        for c in range(NCH):
            V5P[:, (i * NCH + c) * D + NCH * (i % NVP) + c] = V5[:, i]
            VBP[:, (i * NCH + c) * D + NCH * (i % NVP) + c] = VB[:, i]
    return {
        "GW2": _bf16(g_W2),
        "FW2": _bf16(f_W2),
        "G5P": _bf16(G5P),
        "G3D": _fp8(G3D.reshape(A, 2 * 2 * A)),
        "FW1P": _bf16(FW1P),
        "BD": _bf16(BD),
        "U": U, "C1": C1, "C2": C2,
        "GB2": np.ascontiguousarray(g_b2.reshape(A, 1), f),
        "FB2": np.ascontiguousarray(f_b2.reshape(A, 1), f),
        "V5P": _bf16(V5P), "VBP": _bf16(VBP),
    }


def _out_bias(f_b2, f_b3, f_W3):
    # out const: f_b3 + a * f_b2 @ V
    return (f_b3 + AF * (f_b2 @ f_W3)).astype(np.float32)


def _kernel_numpy(X, W, embeddings, g_W1, g_b1, g_W2, g_b2, g_W3, g_b3,
                  f_W1, f_b1, f_W2, f_b2, f_W3, f_b3, group_mask):
    # general fallback (non-identity group_mask)
    def lrelu(x):
        return np.where(x > 0, x, ALPHA * x)

    def mlp(x, W1, b1, W2, b2, W3, b3):
        h = lrelu(x @ W1 + b1)
        h = h + lrelu(h @ W2 + b2)
        return h @ W3 + b3

    n = W.shape[0]
    W_adj = W * (1.0 - np.eye(n, dtype=W.dtype))
    Xm = X[:, None, :] * group_mask
    E = np.broadcast_to(embeddings, (X.shape[0], n, embeddings.shape[1]))
    Xe = mlp(np.concatenate([Xm, E], 2), g_W1, g_b1, g_W2, g_b2, g_W3, g_b3)
    Xa = np.einsum("ji,bjd->bid", W_adj, Xe)
    Xr = mlp(np.concatenate([Xa, E], 2), f_W1, f_b1, f_W2, f_b2, f_W3, f_b3)
    return (Xr * group_mask).sum(axis=1).astype(np.float32)


def kernel(X, W, embeddings, g_W1, g_b1, g_W2, g_b2, g_W3, g_b3,
           f_W1, f_b1, f_W2, f_b2, f_W3, f_b3, group_mask, _run_kw=None):
    if not np.allclose(group_mask, np.eye(N, D, dtype=np.float32)):
        return _kernel_numpy(X, W, embeddings, g_W1, g_b1, g_W2, g_b2, g_W3,
                             g_b3, f_W1, f_b1, f_W2, f_b2, f_W3, f_b3,
                             group_mask)

    from concourse import bass_utils

    consts = _host_consts(W, embeddings, g_W1, g_b1, g_W2, g_b2, g_W3, g_b3,
                          f_W1, f_b1, f_W2, f_b2, f_W3, f_b3)
    XT = _bf16(np.asarray(X, np.float32).T)  # [N, B] bf16
    in_maps = []
    for k in range(NCORES):
        m = dict(consts)
        m["XT"] = np.ascontiguousarray(XT[:, k * BC : (k + 1) * BC])
        in_maps.append(m)

    nc = _get_program()
    res = bass_utils.run_bass_kernel_spmd(
        nc, in_maps, core_ids=list(range(NCORES)), **(_run_kw or {})
    )
    out = np.empty((B, D), np.float32)
    for k in range(NCORES):
        out[k * BC : (k + 1) * BC, :] = res.results[k]["OUT"].T
    out += _out_bias(f_b2, f_b3, f_W3).reshape(1, D)
    if _run_kw:
        kernel.last_results = res
    return out
